# revision 9
# baseline (speedup 1.0000x reference)
"""Trainium2 Bass kernel for nn_Attention (dense transformer block).

Strategy: data-parallel over batch across 8 NeuronCores (8 batches/core).
Per core, per batch (N=256 tokens, 16 heads, dim_head=32):
  - qkv projection: q,k computed TRANSPOSED (qkT [j, n], weight-stationary),
    v computed untransposed (x^T-stationary) -> v [n, j] so the attn@v matmul
    needs no on-chip transposes at all.
  - dots^T[m, n] = k_h^T.T-stationary @ q_h^T streaming, K=32, 4 heads packed
    into the 4 PE row-groups (concurrent matmuls).
  - softmax without max-subtraction (|dots| <~ 1.5 by construction) and with
    normalization deferred: attn_unnorm = exp(dots^T) * exp(bias^T) (exp on
    ACT reading PSUM directly, bias multiply on DVE/GPSIMD in bf16).
  - attn@v: out_h^T[d, n] = v_h[m, d]-stationary @ attn^T streaming, 4 heads
    packed into PE col-groups; a parallel ones[m,32]-stationary matmul
    computes the softmax denominators as a 32-row broadcast, so
    reciprocal+normalize are dense per-partition DVE ops.
  - out projection with b_out folded in as a K=1 matmul row; PSUM -> DRAM DMA.
All matmuls in bf16 (fp32 PSUM accumulation); rel-err vs fp32 reference ~1e-3.
"""

import os
import sys

import numpy as np

if "/opt/trn_rl_repo" not in sys.path:
    sys.path.insert(0, "/opt/trn_rl_repo")

import ml_dtypes  # noqa: E402

from concourse import bass, bacc, mybir  # noqa: E402
from concourse.tile import TileContext  # noqa: E402
from concourse.bass_utils import run_bass_kernel_spmd  # noqa: E402

BF16 = mybir.dt.bfloat16
F32 = mybir.dt.float32
NPBF16 = ml_dtypes.bfloat16

B, N, INP, OUP, H, D = 64, 256, 512, 512, 16, 32
NCORES = 8
BL = B // NCORES  # batches per core
SCALE = D ** -0.5

_CACHE = {}


def _relative_index(ih: int, iw: int) -> np.ndarray:
    yy, xx = np.meshgrid(np.arange(ih), np.arange(iw), indexing="ij")
    coords = np.stack([yy.ravel(), xx.ravel()])
    rel = coords[:, :, None] - coords[:, None, :]
    rel[0] += ih - 1
    rel[1] += iw - 1
    rel[0] *= 2 * iw - 1
    return rel.sum(0).ravel()


def _build(bl: int, repeats: int = 1):
    nc = bacc.Bacc(None, target_bir_lowering=False)

    xT = nc.declare_dram_parameter("xT", [bl, 128, 4, 256], BF16, isOutput=False)
    wqkv = nc.declare_dram_parameter("wqkv", [128, 4, 1536], BF16, isOutput=False)
    w2t = nc.declare_dram_parameter("w2t", [128, 4, 512], BF16, isOutput=False)
    ebT = nc.declare_dram_parameter("ebT", [128, 2, 4096], BF16, isOutput=False)
    bout = nc.declare_dram_parameter("bout", [1, 512], BF16, isOutput=False)
    y = nc.declare_dram_parameter("y", [bl, 2, 128, 512], F32, isOutput=True)

    EXP = mybir.ActivationFunctionType.Exp

    with TileContext(nc) as tc:
        with (
            tc.tile_pool(name="consts", bufs=1) as consts,
            tc.tile_pool(name="xpool", bufs=3) as xpool,
            tc.tile_pool(name="qkvpool", bufs=2) as qkvpool,
            tc.tile_pool(name="attnpool", bufs=2) as attnpool,
            tc.tile_pool(name="small", bufs=4) as small,
            tc.tile_pool(name="pbig", bufs=2, space="PSUM") as pbig,
            tc.tile_pool(name="pod", bufs=2, space="PSUM") as pod,
            tc.tile_pool(name="psmall", bufs=2, space="PSUM") as psmall,
        ):
            wq_sb = consts.tile([128, 4, 1536], BF16)
            nc.sync.dma_start(wq_sb[:], wqkv[:])
            w2_sb = consts.tile([128, 4, 512], BF16)
            nc.sync.dma_start(w2_sb[:], w2t[:])
            eb_sb = consts.tile([128, 2, 4096], BF16)
            nc.sync.dma_start(eb_sb[:], ebT[:])
            bo_sb = consts.tile([1, 512], BF16)
            nc.sync.dma_start(bo_sb[:], bout[:])
            ones32 = consts.tile([128, 32], BF16)
            nc.vector.memset(ones32[:], 1.0)
            ones1 = consts.tile([1, 128], BF16)
            nc.vector.memset(ones1[:], 1.0)
            # zero-padded q staging: q0[p, h, n] nonzero only for
            # p in [32*(h%4), 32*(h%4)+32); the zero rows are written once
            # and never touched again (per-batch DMAs overwrite only the
            # nonzero rows), so the cross-head terms of the full-K dots
            # matmuls vanish. Two buffers, alternated by batch parity.
            q0 = []
            for i in range(2):
                t = consts.tile([128, 16, 256], BF16, name=f"q0_{i}")
                nc.vector.memset(t[:], 0.0)
                q0.append(t)

            for rep in range(repeats):
              for b in range(bl):
                xt = xpool.tile([128, 4, 256], BF16, tag="xt")
                nc.sync.dma_start(xt[:], xT[b])

                qkT = qkvpool.tile([128, 2048], BF16, tag="qkT")
                vt = qkvpool.tile([128, 2, 512], BF16, tag="vt")

                # q,k projection (transposed): out[j, n] over j-tiles 0..7
                for half in range(2):
                    pqk = pbig.tile([128, 1024], F32, tag="pbig")
                    for jq in range(4):
                        jt = half * 4 + jq
                        for it in range(4):
                            nc.tensor.matmul(
                                pqk[:, jq * 256 : (jq + 1) * 256],
                                lhsT=wq_sb[:, it, jt * 128 : (jt + 1) * 128],
                                rhs=xt[:, it, :],
                                start=(it == 0),
                                stop=(it == 3),
                            )
                    nc.vector.tensor_copy(
                        out=qkT[:, half * 1024 : (half + 1) * 1024], in_=pqk[:]
                    )

                # v projection (untransposed): v[n, j]
                for nt in range(2):
                    pv = psmall.tile([128, 512], F32, tag="psmall")
                    for it in range(4):
                        nc.tensor.matmul(
                            pv[:],
                            lhsT=xt[:, it, nt * 128 : (nt + 1) * 128],
                            rhs=wq_sb[:, it, 1024:1536],
                            start=(it == 0),
                            stop=(it == 3),
                        )
                    nc.vector.tensor_copy(out=vt[:, nt, :], in_=pv[:])

                # stage zero-padded q tiles (pure DMA, no engine cost)
                qz = q0[b % 2]
                for h in range(H):
                    hp = h % 4
                    g = h // 4
                    nc.sync.dma_start(
                        out=qz[32 * hp : 32 * (hp + 1), h, :],
                        in_=qkT[32 * hp : 32 * (hp + 1), g * 256 : (g + 1) * 256],
                    )

                # attention scores, exp, bias
                attn = [
                    attnpool.tile(
                        [128, 4096], BF16, tag=f"attn{mt}", name=f"attn{mt}"
                    )
                    for mt in range(2)
                ]
                for g in range(4):
                    for mt in range(2):
                        pd = pbig.tile([128, 1024], F32, tag="pbig")
                        for t in range(2):
                            nc.tensor.matmul(
                                pd[:, t * 512 : (t + 1) * 512],
                                lhsT=qkT[
                                    :,
                                    (4 + g) * 256 + mt * 128 : (4 + g) * 256
                                    + (mt + 1) * 128,
                                ],
                                rhs=qz[:, 4 * g + 2 * t : 4 * g + 2 * t + 2, :],
                                start=True,
                                stop=True,
                            )
                        nc.scalar.activation(
                            out=attn[mt][:, g * 1024 : (g + 1) * 1024],
                            in_=pd[:],
                            func=EXP,
                        )
                        eng = nc.vector if (g % 2 == 0) else nc.gpsimd
                        eng.tensor_mul(
                            attn[mt][:, g * 1024 : (g + 1) * 1024],
                            attn[mt][:, g * 1024 : (g + 1) * 1024],
                            eb_sb[:, mt, g * 1024 : (g + 1) * 1024],
                        )

                # attn @ v (+ denominators via ones-stationary matmuls)
                outT = small.tile([128, 1024], BF16, tag="outT")
                for g in range(4):
                    od = pod.tile([128, 512], F32, tag="pod")
                    for hp in range(4):
                        h = 4 * g + hp
                        for mt in range(2):
                            nc.tensor.matmul(
                                od[32 * hp : 32 * (hp + 1), 0:256],
                                lhsT=vt[:, mt, 32 * h : 32 * h + 32],
                                rhs=attn[mt][:, h * 256 : (h + 1) * 256],
                                start=(mt == 0),
                                stop=(mt == 1),
                                tile_position=(0, 32 * hp),
                            )
                    for hp in range(4):
                        h = 4 * g + hp
                        for mt in range(2):
                            nc.tensor.matmul(
                                od[32 * hp : 32 * (hp + 1), 256:512],
                                lhsT=ones32[:],
                                rhs=attn[mt][:, h * 256 : (h + 1) * 256],
                                start=(mt == 0),
                                stop=(mt == 1),
                                tile_position=(0, 32 * hp),
                            )
                    r = small.tile([128, 256], F32, tag="r")
                    nc.vector.reciprocal_approx_fast(out=r[:], in_=od[:, 256:512])
                    nc.vector.tensor_mul(
                        outT[:, g * 256 : (g + 1) * 256], od[:, 0:256], r[:]
                    )

                # output projection + bias, straight to DRAM from PSUM
                for nt in range(2):
                    py = psmall.tile([128, 512], F32, tag="psmall")
                    for ot in range(4):
                        nc.tensor.matmul(
                            py[:],
                            lhsT=outT[
                                :, ot * 256 + nt * 128 : ot * 256 + nt * 128 + 128
                            ],
                            rhs=w2_sb[:, ot, :],
                            start=(ot == 0),
                            stop=False,
                        )
                    nc.tensor.matmul(
                        py[:], lhsT=ones1[:], rhs=bo_sb[:], start=False, stop=True
                    )
                    ysb = small.tile([128, 512], F32, tag="ysb", name="ysb")
                    if nt == 0:
                        nc.vector.tensor_copy(out=ysb[:], in_=py[:])
                    else:
                        nc.scalar.copy(out=ysb[:], in_=py[:])
                    nc.sync.dma_start(out=y[b, nt], in_=ysb[:])

    nc.compile()
    return nc


def _get_nc(bl: int, repeats: int = 1):
    key = (bl, repeats)
    if key not in _CACHE:
        _CACHE[key] = _build(bl, repeats)
    return _CACHE[key]


def _prep_inputs(x, w_qkv, rel_bias_table, w_out, b_out):
    """Host-side layout prep: transpose/tile/bf16-cast, bias-table gather."""
    x = np.asarray(x, np.float32)
    w_qkv = np.asarray(w_qkv, np.float32).copy()
    rel_bias_table = np.asarray(rel_bias_table, np.float32)
    w_out = np.asarray(w_out, np.float32)
    b_out = np.asarray(b_out, np.float32)

    # fold the attention scale into the q columns of w_qkv
    w_qkv[:, :OUP] *= SCALE

    # xT_dev[b, p, it, n] = x[b, n, it*128+p]
    xT = np.ascontiguousarray(
        x.transpose(0, 2, 1).reshape(B, 4, 128, N).transpose(0, 2, 1, 3)
    ).astype(NPBF16)
    # wqkv_dev[p, it, j] = w_qkv[it*128+p, j]
    wqkv_dev = np.ascontiguousarray(
        w_qkv.reshape(4, 128, 3 * OUP).transpose(1, 0, 2)
    ).astype(NPBF16)
    # w2t_dev[p, ot, q] = w_out.T[ot*128+p, q] = w_out[q, ot*128+p]
    w2t_dev = np.ascontiguousarray(
        w_out.T.reshape(4, 128, OUP).transpose(1, 0, 2)
    ).astype(NPBF16)
    # bias[n, m, h]; ebT_dev[p, mt, h*256+n] = exp(bias[n, mt*128+p, h])
    rel_idx = _relative_index(16, 16)
    bias = rel_bias_table[rel_idx].reshape(N, N, H)  # [n, m, h]
    ebT = np.exp(bias.transpose(2, 1, 0))  # [h, m, n]
    ebT_dev = np.ascontiguousarray(
        ebT.reshape(H, 2, 128, N).transpose(2, 1, 0, 3).reshape(128, 2, H * N)
    ).astype(NPBF16)
    bout_dev = b_out.reshape(1, OUP).astype(NPBF16)
    return xT, wqkv_dev, w2t_dev, ebT_dev, bout_dev


def kernel(x, w_qkv, rel_bias_table, w_out, b_out, ih, iw):
    assert int(ih) == 16 and int(iw) == 16
    xT, wqkv_dev, w2t_dev, ebT_dev, bout_dev = _prep_inputs(
        x, w_qkv, rel_bias_table, w_out, b_out
    )

    nc = _get_nc(BL)
    in_maps = []
    for c in range(NCORES):
        in_maps.append(
            {
                "xT": np.ascontiguousarray(xT[c * BL : (c + 1) * BL]),
                "wqkv": wqkv_dev,
                "w2t": w2t_dev,
                "ebT": ebT_dev,
                "bout": bout_dev,
            }
        )

    trace = bool(os.environ.get("BASS_TRACE_KERNEL"))
    if trace:
        try:
            from antenv.axon_hooks import get_axon_ntff_profile_hook  # noqa: F401
        except ImportError:
            trace = False
    res = run_bass_kernel_spmd(nc, in_maps, core_ids=list(range(NCORES)), trace=trace)
    kernel.last_result = res
    if res.exec_time_ns is not None:
        print(f"HW exec time: {res.exec_time_ns} ns")

    y = np.concatenate(
        [r["y"].reshape(BL, N, OUP) for r in res.results], axis=0
    ).astype(np.float32)
    return y


kernel.last_result = None


# revision 24
# speedup vs baseline: 1.1509x; 1.1509x over previous
"""Trainium2 Bass kernel for nn_Attention (dense transformer block).

Strategy: data-parallel over batch across 8 NeuronCores (8 batches/core).
Per core, per batch (N=256 tokens, 16 heads, dim_head=32):
  - qkv projection: q,k computed TRANSPOSED (qkT [j, n], weight-stationary),
    v computed untransposed (x^T-stationary) -> v [n, j] so the attn@v matmul
    needs no on-chip transposes at all.
  - dots^T[m, n] = k_h^T.T-stationary @ q_h^T streaming, K=32, 4 heads packed
    into the 4 PE row-groups (concurrent matmuls).
  - softmax without max-subtraction (|dots| <~ 1.5 by construction) and with
    normalization deferred: attn_unnorm = exp(dots^T) * exp(bias^T) (exp on
    ACT reading PSUM directly, bias multiply on DVE/GPSIMD in bf16).
  - attn@v: out_h^T[d, n] = v_h[m, d]-stationary @ attn^T streaming, 4 heads
    packed into PE col-groups; a parallel ones[m,32]-stationary matmul
    computes the softmax denominators as a 32-row broadcast, so
    reciprocal+normalize are dense per-partition DVE ops.
  - out projection with b_out folded in as a K=1 matmul row; PSUM -> DRAM DMA.
All matmuls in bf16 (fp32 PSUM accumulation); rel-err vs fp32 reference ~1e-3.
"""

import os
import sys

import numpy as np

if "/opt/trn_rl_repo" not in sys.path:
    sys.path.insert(0, "/opt/trn_rl_repo")

import ml_dtypes  # noqa: E402

from concourse import bacc, mybir  # noqa: E402
from concourse.tile import TileContext  # noqa: E402
from concourse.bass_utils import run_bass_kernel_spmd  # noqa: E402

BF16 = mybir.dt.bfloat16
F32 = mybir.dt.float32
NPBF16 = ml_dtypes.bfloat16

B, N, INP, OUP, H, D = 64, 256, 512, 512, 16, 32
NCORES = 8
BL = B // NCORES  # batches per core
SCALE = D ** -0.5

_CACHE = {}


def _relative_index(ih: int, iw: int) -> np.ndarray:
    yy, xx = np.meshgrid(np.arange(ih), np.arange(iw), indexing="ij")
    coords = np.stack([yy.ravel(), xx.ravel()])
    rel = coords[:, :, None] - coords[:, None, :]
    rel[0] += ih - 1
    rel[1] += iw - 1
    rel[0] *= 2 * iw - 1
    return rel.sum(0).ravel()


DEFAULT_OPTS = {
    "bias_dve_mod": 3,      # (2g+mt) % 8 < this -> DVE, else GPSIMD
    "q0_merged_dma": False,  # 4 merged q0 DMAs vs 16 per-head
    "pbig_bufs": 2,
    "pod_bufs": 2,
    "psmall_bufs": 2,
    "xpool_bufs": 3,
    "qkv_bufs": 2,
    "attn_bufs": 2,
    "evac_chunks": 2,
}


def _build(bl: int, repeats: int = 1, opts: dict | None = None):
    o = dict(DEFAULT_OPTS)
    if opts:
        o.update(opts)
    nc = bacc.Bacc(None, target_bir_lowering=False)

    xT = nc.declare_dram_parameter("xT", [bl, 128, 4, 256], BF16, isOutput=False)
    wqkv = nc.declare_dram_parameter("wqkv", [128, 4, 1536], BF16, isOutput=False)
    w2t = nc.declare_dram_parameter("w2t", [128, 4, 512], BF16, isOutput=False)
    ebT = nc.declare_dram_parameter("ebT", [128, 2, 4096], BF16, isOutput=False)
    bout = nc.declare_dram_parameter("bout", [1, 512], BF16, isOutput=False)
    y = nc.declare_dram_parameter("y", [bl, 2, 128, 512], F32, isOutput=True)

    EXP = mybir.ActivationFunctionType.Exp

    with TileContext(nc) as tc:
        with (
            tc.tile_pool(name="consts", bufs=1) as consts,
            tc.tile_pool(name="xpool", bufs=o["xpool_bufs"]) as xpool,
            tc.tile_pool(name="qkvpool", bufs=o["qkv_bufs"]) as qkvpool,
            tc.tile_pool(name="attnpool", bufs=o["attn_bufs"]) as attnpool,
            tc.tile_pool(name="small", bufs=4) as small,
            tc.tile_pool(name="pbig", bufs=o["pbig_bufs"], space="PSUM") as pbig,
            tc.tile_pool(name="pod", bufs=o["pod_bufs"], space="PSUM") as pod,
            tc.tile_pool(name="psmall", bufs=o["psmall_bufs"], space="PSUM") as psmall,
        ):
            wq_sb = consts.tile([128, 4, 1536], BF16)
            for it in range(4):
                nc.sync.dma_start(wq_sb[:, it, :], wqkv[:, it, :])
            w2_sb = consts.tile([128, 4, 512], BF16)
            nc.scalar.dma_start(w2_sb[:], w2t[:])
            eb_sb = consts.tile([128, 2, 4096], BF16)
            nc.gpsimd.dma_start(eb_sb[:], ebT[:])
            bo_sb = consts.tile([1, 512], BF16)
            nc.sync.dma_start(bo_sb[:], bout[:])
            ones32 = consts.tile([128, 32], BF16)
            nc.vector.memset(ones32[:], 1.0)
            ones1 = consts.tile([1, 128], BF16)
            nc.vector.memset(ones1[:], 1.0)
            # zero-padded q staging: q0[p, h, n] nonzero only for
            # p in [32*(h%4), 32*(h%4)+32); the zero rows are written once
            # and never touched again (per-batch DMAs overwrite only the
            # nonzero rows), so the cross-head terms of the full-K dots
            # matmuls vanish. Two buffers, alternated by batch parity.
            q0 = []
            for i in range(2):
                t = consts.tile([128, 16, 256], BF16, name=f"q0_{i}")
                nc.vector.memset(t[:], 0.0)
                q0.append(t)

            for rep in range(repeats):
              for b in range(bl):
                xt = xpool.tile([128, 4, 256], BF16, tag="xt")
                nc.sync.dma_start(xt[:], xT[b])

                qkT = qkvpool.tile([128, 2048], BF16, tag="qkT")
                vt = qkvpool.tile([128, 2, 512], BF16, tag="vt")

                # q,k projection (transposed): out[j, n] over j-tiles 0..7
                for half in range(2):
                    pqk = pbig.tile([128, 1024], F32, tag="pbig")
                    for jq in range(4):
                        jt = half * 4 + jq
                        for it in range(4):
                            nc.tensor.matmul(
                                pqk[:, jq * 256 : (jq + 1) * 256],
                                lhsT=wq_sb[:, it, jt * 128 : (jt + 1) * 128],
                                rhs=xt[:, it, :],
                                start=(it == 0),
                                stop=(it == 3),
                            )
                    if o["evac_chunks"] == 1 or half == 1:
                        nc.vector.tensor_copy(
                            out=qkT[:, half * 1024 : (half + 1) * 1024], in_=pqk[:]
                        )
                    else:
                        ch = 1024 // o["evac_chunks"]
                        for ci in range(o["evac_chunks"]):
                            nc.vector.tensor_copy(
                                out=qkT[:, ci * ch : (ci + 1) * ch],
                                in_=pqk[:, ci * ch : (ci + 1) * ch],
                            )

                # v projection (untransposed): v[n, j]
                for nt in range(2):
                    pv = psmall.tile([128, 512], F32, tag="psmall")
                    for it in range(4):
                        nc.tensor.matmul(
                            pv[:],
                            lhsT=xt[:, it, nt * 128 : (nt + 1) * 128],
                            rhs=wq_sb[:, it, 1024:1536],
                            start=(it == 0),
                            stop=(it == 3),
                        )
                    nc.vector.tensor_copy(out=vt[:, nt, :], in_=pv[:])

                # stage zero-padded q tiles (pure DMA, no engine cost);
                # one DMA per hp covers all four groups g: head h = 4g+hp
                # lives at partitions [32hp, 32hp+32), dst slot h, src block g.
                qz = q0[b % 2]
                if o["q0_merged_dma"] == "8way":
                    qz_g = qz.rearrange("p (g q) n -> p g q n", q=4)
                    for hp in range(4):
                        for gh in range(2):
                            nc.sync.dma_start(
                                out=qz_g[
                                    32 * hp : 32 * (hp + 1), 2 * gh : 2 * gh + 2, hp, :
                                ],
                                in_=qkT[
                                    32 * hp : 32 * (hp + 1),
                                    512 * gh : 512 * (gh + 1),
                                ].rearrange("p (g n) -> p g n", n=256),
                            )
                elif o["q0_merged_dma"]:
                    qz_g = qz.rearrange("p (g q) n -> p g q n", q=4)
                    for hp in range(4):
                        nc.sync.dma_start(
                            out=qz_g[32 * hp : 32 * (hp + 1), :, hp, :],
                            in_=qkT[32 * hp : 32 * (hp + 1), 0:1024].rearrange(
                                "p (g n) -> p g n", n=256
                            ),
                        )
                else:
                    for h in range(H):
                        hp, g = h % 4, h // 4
                        nc.sync.dma_start(
                            out=qz[32 * hp : 32 * (hp + 1), h, :],
                            in_=qkT[32 * hp : 32 * (hp + 1), g * 256 : (g + 1) * 256],
                        )

                # attention scores, exp, bias
                attn = [
                    attnpool.tile(
                        [128, 4096], BF16, tag=f"attn{mt}", name=f"attn{mt}"
                    )
                    for mt in range(2)
                ]
                for g in range(4):
                    for mt in range(2):
                        pd = pbig.tile([128, 1024], F32, tag="pbig")
                        for t in range(2):
                            nc.tensor.matmul(
                                pd[:, t * 512 : (t + 1) * 512],
                                lhsT=qkT[
                                    :,
                                    (4 + g) * 256 + mt * 128 : (4 + g) * 256
                                    + (mt + 1) * 128,
                                ],
                                rhs=qz[:, 4 * g + 2 * t : 4 * g + 2 * t + 2, :],
                                start=True,
                                stop=True,
                            )
                        nc.scalar.activation(
                            out=attn[mt][:, g * 1024 : (g + 1) * 1024],
                            in_=pd[:],
                            func=EXP,
                        )
                        eng = nc.vector if (2 * g + mt) % 8 < o["bias_dve_mod"] else nc.gpsimd
                        eng.tensor_mul(
                            attn[mt][:, g * 1024 : (g + 1) * 1024],
                            attn[mt][:, g * 1024 : (g + 1) * 1024],
                            eb_sb[:, mt, g * 1024 : (g + 1) * 1024],
                        )

                # attn @ v (+ denominators via ones-stationary matmuls)
                outT = small.tile([128, 1024], BF16, tag="outT")
                for g in range(4):
                    od = pod.tile([128, 512], F32, tag="pod")
                    for hp in range(4):
                        h = 4 * g + hp
                        for mt in range(2):
                            nc.tensor.matmul(
                                od[32 * hp : 32 * (hp + 1), 0:256],
                                lhsT=vt[:, mt, 32 * h : 32 * h + 32],
                                rhs=attn[mt][:, h * 256 : (h + 1) * 256],
                                start=(mt == 0),
                                stop=(mt == 1),
                                tile_position=(0, 32 * hp),
                            )
                    for hp in range(4):
                        h = 4 * g + hp
                        for mt in range(2):
                            nc.tensor.matmul(
                                od[32 * hp : 32 * (hp + 1), 256:512],
                                lhsT=ones32[:],
                                rhs=attn[mt][:, h * 256 : (h + 1) * 256],
                                start=(mt == 0),
                                stop=(mt == 1),
                                tile_position=(0, 32 * hp),
                            )
                    r = small.tile([128, 256], F32, tag="r")
                    nc.vector.reciprocal_approx_fast(out=r[:], in_=od[:, 256:512])
                    nc.vector.tensor_mul(
                        outT[:, g * 256 : (g + 1) * 256], od[:, 0:256], r[:]
                    )

                # output projection + bias, straight to DRAM from PSUM
                for nt in range(2):
                    py = psmall.tile([128, 512], F32, tag="psmall")
                    for ot in range(4):
                        nc.tensor.matmul(
                            py[:],
                            lhsT=outT[
                                :, ot * 256 + nt * 128 : ot * 256 + nt * 128 + 128
                            ],
                            rhs=w2_sb[:, ot, :],
                            start=(ot == 0),
                            stop=False,
                        )
                    nc.tensor.matmul(
                        py[:], lhsT=ones1[:], rhs=bo_sb[:], start=False, stop=True
                    )
                    ysb = small.tile([128, 512], F32, tag="ysb", name="ysb")
                    if nt == 0:
                        nc.vector.tensor_copy(out=ysb[:], in_=py[:])
                    else:
                        nc.scalar.copy(out=ysb[:], in_=py[:])
                    nc.sync.dma_start(out=y[b, nt], in_=ysb[:])

    nc.compile()
    return nc


def _get_nc(bl: int, repeats: int = 1, opts: dict | None = None):
    key = (bl, repeats, tuple(sorted((opts or {}).items())))
    if key not in _CACHE:
        _CACHE[key] = _build(bl, repeats, opts)
    return _CACHE[key]


def _prep_inputs(x, w_qkv, rel_bias_table, w_out, b_out):
    """Host-side layout prep: transpose/tile/bf16-cast, bias-table gather."""
    x = np.asarray(x, np.float32)
    w_qkv = np.asarray(w_qkv, np.float32).copy()
    rel_bias_table = np.asarray(rel_bias_table, np.float32)
    w_out = np.asarray(w_out, np.float32)
    b_out = np.asarray(b_out, np.float32)

    # fold the attention scale into the q columns of w_qkv
    w_qkv[:, :OUP] *= SCALE

    # xT_dev[b, p, it, n] = x[b, n, it*128+p]
    xT = np.ascontiguousarray(
        x.transpose(0, 2, 1).reshape(B, 4, 128, N).transpose(0, 2, 1, 3)
    ).astype(NPBF16)
    # wqkv_dev[p, it, j] = w_qkv[it*128+p, j]
    wqkv_dev = np.ascontiguousarray(
        w_qkv.reshape(4, 128, 3 * OUP).transpose(1, 0, 2)
    ).astype(NPBF16)
    # w2t_dev[p, ot, q] = w_out.T[ot*128+p, q] = w_out[q, ot*128+p]
    w2t_dev = np.ascontiguousarray(
        w_out.T.reshape(4, 128, OUP).transpose(1, 0, 2)
    ).astype(NPBF16)
    # bias[n, m, h]; ebT_dev[p, mt, h*256+n] = exp(bias[n, mt*128+p, h])
    rel_idx = _relative_index(16, 16)
    bias = rel_bias_table[rel_idx].reshape(N, N, H)  # [n, m, h]
    ebT = np.exp(bias.transpose(2, 1, 0))  # [h, m, n]
    ebT_dev = np.ascontiguousarray(
        ebT.reshape(H, 2, 128, N).transpose(2, 1, 0, 3).reshape(128, 2, H * N)
    ).astype(NPBF16)
    bout_dev = b_out.reshape(1, OUP).astype(NPBF16)
    return xT, wqkv_dev, w2t_dev, ebT_dev, bout_dev


def kernel(x, w_qkv, rel_bias_table, w_out, b_out, ih, iw):
    assert int(ih) == 16 and int(iw) == 16
    xT, wqkv_dev, w2t_dev, ebT_dev, bout_dev = _prep_inputs(
        x, w_qkv, rel_bias_table, w_out, b_out
    )

    nc = _get_nc(BL)
    in_maps = []
    for c in range(NCORES):
        in_maps.append(
            {
                "xT": np.ascontiguousarray(xT[c * BL : (c + 1) * BL]),
                "wqkv": wqkv_dev,
                "w2t": w2t_dev,
                "ebT": ebT_dev,
                "bout": bout_dev,
            }
        )

    trace = bool(os.environ.get("BASS_TRACE_KERNEL"))
    if trace:
        try:
            from antenv.axon_hooks import get_axon_ntff_profile_hook  # noqa: F401
        except ImportError:
            trace = False
    res = run_bass_kernel_spmd(nc, in_maps, core_ids=list(range(NCORES)), trace=trace)
    kernel.last_result = res
    if res.exec_time_ns is not None:
        print(f"HW exec time: {res.exec_time_ns} ns")

    y = np.concatenate(
        [r["y"].reshape(BL, N, OUP) for r in res.results], axis=0
    ).astype(np.float32)
    return y


kernel.last_result = None


# revision 25
# speedup vs baseline: 2.4733x; 2.1489x over previous
"""Trainium2 Bass kernel for nn_Attention (dense transformer block).

Strategy: data-parallel over batch across 8 NeuronCores (8 batches/core).
Per core, per batch (N=256 tokens, 16 heads, dim_head=32):
  - qkv projection: q,k computed TRANSPOSED (qkT [j, n], weight-stationary),
    v computed untransposed (x^T-stationary) -> v [n, j] so the attn@v matmul
    needs no on-chip transposes at all.
  - dots^T[m, n] = k_h^T.T-stationary @ q_h^T streaming, K=32, 4 heads packed
    into the 4 PE row-groups (concurrent matmuls).
  - softmax without max-subtraction (|dots| <~ 1.5 by construction) and with
    normalization deferred: attn_unnorm = exp(dots^T) * exp(bias^T) (exp on
    ACT reading PSUM directly, bias multiply on DVE/GPSIMD in bf16).
  - attn@v: out_h^T[d, n] = v_h[m, d]-stationary @ attn^T streaming, 4 heads
    packed into PE col-groups; a parallel ones[m,32]-stationary matmul
    computes the softmax denominators as a 32-row broadcast, so
    reciprocal+normalize are dense per-partition DVE ops.
  - out projection with b_out folded in as a K=1 matmul row; PSUM -> DRAM DMA.
All matmuls in bf16 (fp32 PSUM accumulation); rel-err vs fp32 reference ~1e-3.
"""

import os
import sys

import numpy as np

if "/opt/trn_rl_repo" not in sys.path:
    sys.path.insert(0, "/opt/trn_rl_repo")

import ml_dtypes  # noqa: E402

from concourse import bacc, mybir  # noqa: E402
from concourse.tile import TileContext  # noqa: E402
from concourse.bass_utils import run_bass_kernel_spmd  # noqa: E402

BF16 = mybir.dt.bfloat16
F32 = mybir.dt.float32
NPBF16 = ml_dtypes.bfloat16

B, N, INP, OUP, H, D = 64, 256, 512, 512, 16, 32
NCORES = 8
BL = B // NCORES  # batches per core
SCALE = D ** -0.5

_CACHE = {}


def _relative_index(ih: int, iw: int) -> np.ndarray:
    yy, xx = np.meshgrid(np.arange(ih), np.arange(iw), indexing="ij")
    coords = np.stack([yy.ravel(), xx.ravel()])
    rel = coords[:, :, None] - coords[:, None, :]
    rel[0] += ih - 1
    rel[1] += iw - 1
    rel[0] *= 2 * iw - 1
    return rel.sum(0).ravel()


DEFAULT_OPTS = {
    "bias_dve_mod": 3,      # (2g+mt) % 8 < this -> DVE, else GPSIMD
    "q0_merged_dma": False,  # 4 merged q0 DMAs vs 16 per-head
    "pbig_bufs": 2,
    "pod_bufs": 2,
    "psmall_bufs": 2,
    "xpool_bufs": 3,
    "qkv_bufs": 2,
    "attn_bufs": 2,
    "evac_chunks": 2,
    "v_after_dots": False,
}


def _build(bl: int, repeats: int = 1, opts: dict | None = None):
    o = dict(DEFAULT_OPTS)
    if opts:
        o.update(opts)
    nc = bacc.Bacc(None, target_bir_lowering=False)

    xT = nc.declare_dram_parameter("xT", [bl, 128, 4, 256], BF16, isOutput=False)
    wqkv = nc.declare_dram_parameter("wqkv", [128, 4, 1536], BF16, isOutput=False)
    w2t = nc.declare_dram_parameter("w2t", [128, 4, 512], BF16, isOutput=False)
    ebT = nc.declare_dram_parameter("ebT", [128, 2, 4096], BF16, isOutput=False)
    bout = nc.declare_dram_parameter("bout", [1, 512], BF16, isOutput=False)
    y = nc.declare_dram_parameter("y", [bl, 2, 128, 512], F32, isOutput=True)

    EXP = mybir.ActivationFunctionType.Exp

    with TileContext(nc) as tc:
        with (
            tc.tile_pool(name="consts", bufs=1) as consts,
            tc.tile_pool(name="xpool", bufs=o["xpool_bufs"]) as xpool,
            tc.tile_pool(name="qkvpool", bufs=o["qkv_bufs"]) as qkvpool,
            tc.tile_pool(name="attnpool", bufs=o["attn_bufs"]) as attnpool,
            tc.tile_pool(name="small", bufs=4) as small,
            tc.tile_pool(name="pbig", bufs=o["pbig_bufs"], space="PSUM") as pbig,
            tc.tile_pool(name="pod", bufs=o["pod_bufs"], space="PSUM") as pod,
            tc.tile_pool(name="psmall", bufs=o["psmall_bufs"], space="PSUM") as psmall,
        ):
            wq_sb = consts.tile([128, 4, 1536], BF16)
            for it in range(4):
                nc.sync.dma_start(wq_sb[:, it, :], wqkv[:, it, :])
            w2_sb = consts.tile([128, 4, 512], BF16)
            nc.scalar.dma_start(w2_sb[:], w2t[:])
            eb_sb = consts.tile([128, 2, 4096], BF16)
            nc.gpsimd.dma_start(eb_sb[:], ebT[:])
            bo_sb = consts.tile([1, 512], BF16)
            nc.sync.dma_start(bo_sb[:], bout[:])
            ones32 = consts.tile([128, 32], BF16)
            nc.vector.memset(ones32[:], 1.0)
            ones1 = consts.tile([1, 128], BF16)
            nc.vector.memset(ones1[:], 1.0)
            # zero-padded q staging: q0[p, h, n] nonzero only for
            # p in [32*(h%4), 32*(h%4)+32); the zero rows are written once
            # and never touched again (per-batch DMAs overwrite only the
            # nonzero rows), so the cross-head terms of the full-K dots
            # matmuls vanish. Two buffers, alternated by batch parity.
            q0 = []
            for i in range(2):
                t = consts.tile([128, 16, 256], BF16, name=f"q0_{i}")
                nc.vector.memset(t[:], 0.0)
                q0.append(t)

            for rep in range(repeats):
              for b in range(bl):
                xt = xpool.tile([128, 4, 256], BF16, tag="xt")
                nc.sync.dma_start(xt[:], xT[b])

                qkT = qkvpool.tile([128, 2048], BF16, tag="qkT")
                vt = qkvpool.tile([128, 2, 512], BF16, tag="vt")

                # q,k projection (transposed): out[j, n] over j-tiles 0..7
                for half in range(2):
                    pqk = pbig.tile([128, 1024], F32, tag="pbig")
                    for jq in range(4):
                        jt = half * 4 + jq
                        for it in range(4):
                            nc.tensor.matmul(
                                pqk[:, jq * 256 : (jq + 1) * 256],
                                lhsT=wq_sb[:, it, jt * 128 : (jt + 1) * 128],
                                rhs=xt[:, it, :],
                                start=(it == 0),
                                stop=(it == 3),
                            )
                    if o["evac_chunks"] == 1 or half == 1:
                        nc.vector.tensor_copy(
                            out=qkT[:, half * 1024 : (half + 1) * 1024], in_=pqk[:]
                        )
                    else:
                        ch = 1024 // o["evac_chunks"]
                        for ci in range(o["evac_chunks"]):
                            nc.vector.tensor_copy(
                                out=qkT[:, ci * ch : (ci + 1) * ch],
                                in_=pqk[:, ci * ch : (ci + 1) * ch],
                            )

                def emit_v():
                    # v projection (untransposed): v[n, j]
                    for nt in range(2):
                        pv = psmall.tile([128, 512], F32, tag="psmall", name="pv")
                        for it in range(4):
                            nc.tensor.matmul(
                                pv[:],
                                lhsT=xt[:, it, nt * 128 : (nt + 1) * 128],
                                rhs=wq_sb[:, it, 1024:1536],
                                start=(it == 0),
                                stop=(it == 3),
                            )
                        nc.vector.tensor_copy(out=vt[:, nt, :], in_=pv[:])

                if not o["v_after_dots"]:
                    emit_v()

                # stage zero-padded q tiles (pure DMA, no engine cost);
                # one DMA per hp covers all four groups g: head h = 4g+hp
                # lives at partitions [32hp, 32hp+32), dst slot h, src block g.
                qz = q0[b % 2]
                if o["q0_merged_dma"] == "8way":
                    qz_g = qz.rearrange("p (g q) n -> p g q n", q=4)
                    for hp in range(4):
                        for gh in range(2):
                            nc.sync.dma_start(
                                out=qz_g[
                                    32 * hp : 32 * (hp + 1), 2 * gh : 2 * gh + 2, hp, :
                                ],
                                in_=qkT[
                                    32 * hp : 32 * (hp + 1),
                                    512 * gh : 512 * (gh + 1),
                                ].rearrange("p (g n) -> p g n", n=256),
                            )
                elif o["q0_merged_dma"]:
                    qz_g = qz.rearrange("p (g q) n -> p g q n", q=4)
                    for hp in range(4):
                        nc.sync.dma_start(
                            out=qz_g[32 * hp : 32 * (hp + 1), :, hp, :],
                            in_=qkT[32 * hp : 32 * (hp + 1), 0:1024].rearrange(
                                "p (g n) -> p g n", n=256
                            ),
                        )
                else:
                    for h in range(H):
                        hp, g = h % 4, h // 4
                        nc.sync.dma_start(
                            out=qz[32 * hp : 32 * (hp + 1), h, :],
                            in_=qkT[32 * hp : 32 * (hp + 1), g * 256 : (g + 1) * 256],
                        )

                # attention scores, exp, bias
                attn = [
                    attnpool.tile(
                        [128, 4096], BF16, tag=f"attn{mt}", name=f"attn{mt}"
                    )
                    for mt in range(2)
                ]
                for g in range(4):
                    for mt in range(2):
                        pd = pbig.tile([128, 1024], F32, tag="pbig")
                        for t in range(2):
                            nc.tensor.matmul(
                                pd[:, t * 512 : (t + 1) * 512],
                                lhsT=qkT[
                                    :,
                                    (4 + g) * 256 + mt * 128 : (4 + g) * 256
                                    + (mt + 1) * 128,
                                ],
                                rhs=qz[:, 4 * g + 2 * t : 4 * g + 2 * t + 2, :],
                                start=True,
                                stop=True,
                            )
                        nc.scalar.activation(
                            out=attn[mt][:, g * 1024 : (g + 1) * 1024],
                            in_=pd[:],
                            func=EXP,
                        )
                        eng = nc.vector if (2 * g + mt) % 8 < o["bias_dve_mod"] else nc.gpsimd
                        eng.tensor_mul(
                            attn[mt][:, g * 1024 : (g + 1) * 1024],
                            attn[mt][:, g * 1024 : (g + 1) * 1024],
                            eb_sb[:, mt, g * 1024 : (g + 1) * 1024],
                        )

                if o["v_after_dots"]:
                    emit_v()

                # attn @ v (+ denominators via ones-stationary matmuls)
                outT = small.tile([128, 1024], BF16, tag="outT")
                for g in range(4):
                    od = pod.tile([128, 512], F32, tag="pod")
                    for hp in range(4):
                        h = 4 * g + hp
                        for mt in range(2):
                            nc.tensor.matmul(
                                od[32 * hp : 32 * (hp + 1), 0:256],
                                lhsT=vt[:, mt, 32 * h : 32 * h + 32],
                                rhs=attn[mt][:, h * 256 : (h + 1) * 256],
                                start=(mt == 0),
                                stop=(mt == 1),
                                tile_position=(0, 32 * hp),
                            )
                    for hp in range(4):
                        h = 4 * g + hp
                        for mt in range(2):
                            nc.tensor.matmul(
                                od[32 * hp : 32 * (hp + 1), 256:512],
                                lhsT=ones32[:],
                                rhs=attn[mt][:, h * 256 : (h + 1) * 256],
                                start=(mt == 0),
                                stop=(mt == 1),
                                tile_position=(0, 32 * hp),
                            )
                    r = small.tile([128, 256], F32, tag="r")
                    nc.vector.reciprocal_approx_fast(out=r[:], in_=od[:, 256:512])
                    nc.vector.tensor_mul(
                        outT[:, g * 256 : (g + 1) * 256], od[:, 0:256], r[:]
                    )

                # output projection + bias, straight to DRAM from PSUM
                for nt in range(2):
                    py = psmall.tile([128, 512], F32, tag="psmall")
                    for ot in range(4):
                        nc.tensor.matmul(
                            py[:],
                            lhsT=outT[
                                :, ot * 256 + nt * 128 : ot * 256 + nt * 128 + 128
                            ],
                            rhs=w2_sb[:, ot, :],
                            start=(ot == 0),
                            stop=False,
                        )
                    nc.tensor.matmul(
                        py[:], lhsT=ones1[:], rhs=bo_sb[:], start=False, stop=True
                    )
                    ysb = small.tile([128, 512], F32, tag="ysb", name="ysb")
                    if nt == 0:
                        nc.vector.tensor_copy(out=ysb[:], in_=py[:])
                    else:
                        nc.scalar.copy(out=ysb[:], in_=py[:])
                    nc.sync.dma_start(out=y[b, nt], in_=ysb[:])

    nc.compile()
    return nc


def _get_nc(bl: int, repeats: int = 1, opts: dict | None = None):
    key = (bl, repeats, tuple(sorted((opts or {}).items())))
    if key not in _CACHE:
        _CACHE[key] = _build(bl, repeats, opts)
    return _CACHE[key]


def _prep_inputs(x, w_qkv, rel_bias_table, w_out, b_out):
    """Host-side layout prep: transpose/tile/bf16-cast, bias-table gather."""
    x = np.asarray(x, np.float32)
    w_qkv = np.asarray(w_qkv, np.float32).copy()
    rel_bias_table = np.asarray(rel_bias_table, np.float32)
    w_out = np.asarray(w_out, np.float32)
    b_out = np.asarray(b_out, np.float32)

    # fold the attention scale into the q columns of w_qkv
    w_qkv[:, :OUP] *= SCALE

    # xT_dev[b, p, it, n] = x[b, n, it*128+p]
    xT = np.ascontiguousarray(
        x.transpose(0, 2, 1).reshape(B, 4, 128, N).transpose(0, 2, 1, 3)
    ).astype(NPBF16)
    # wqkv_dev[p, it, j] = w_qkv[it*128+p, j]
    wqkv_dev = np.ascontiguousarray(
        w_qkv.reshape(4, 128, 3 * OUP).transpose(1, 0, 2)
    ).astype(NPBF16)
    # w2t_dev[p, ot, q] = w_out.T[ot*128+p, q] = w_out[q, ot*128+p]
    w2t_dev = np.ascontiguousarray(
        w_out.T.reshape(4, 128, OUP).transpose(1, 0, 2)
    ).astype(NPBF16)
    # bias[n, m, h]; ebT_dev[p, mt, h*256+n] = exp(bias[n, mt*128+p, h])
    rel_idx = _relative_index(16, 16)
    bias = rel_bias_table[rel_idx].reshape(N, N, H)  # [n, m, h]
    ebT = np.exp(bias.transpose(2, 1, 0))  # [h, m, n]
    ebT_dev = np.ascontiguousarray(
        ebT.reshape(H, 2, 128, N).transpose(2, 1, 0, 3).reshape(128, 2, H * N)
    ).astype(NPBF16)
    bout_dev = b_out.reshape(1, OUP).astype(NPBF16)
    return xT, wqkv_dev, w2t_dev, ebT_dev, bout_dev


def kernel(x, w_qkv, rel_bias_table, w_out, b_out, ih, iw):
    assert int(ih) == 16 and int(iw) == 16
    xT, wqkv_dev, w2t_dev, ebT_dev, bout_dev = _prep_inputs(
        x, w_qkv, rel_bias_table, w_out, b_out
    )

    nc = _get_nc(BL)
    in_maps = []
    for c in range(NCORES):
        in_maps.append(
            {
                "xT": np.ascontiguousarray(xT[c * BL : (c + 1) * BL]),
                "wqkv": wqkv_dev,
                "w2t": w2t_dev,
                "ebT": ebT_dev,
                "bout": bout_dev,
            }
        )

    trace = bool(os.environ.get("BASS_TRACE_KERNEL"))
    if trace:
        try:
            from antenv.axon_hooks import get_axon_ntff_profile_hook  # noqa: F401
        except ImportError:
            trace = False
    res = run_bass_kernel_spmd(nc, in_maps, core_ids=list(range(NCORES)), trace=trace)
    kernel.last_result = res
    if res.exec_time_ns is not None:
        print(f"HW exec time: {res.exec_time_ns} ns")

    y = np.concatenate(
        [r["y"].reshape(BL, N, OUP) for r in res.results], axis=0
    ).astype(np.float32)
    return y


kernel.last_result = None
